# revision 25
# baseline (speedup 1.0000x reference)
"""Causal GQA self-attention (b=2, t=2048, 16 q-heads / 4 kv-heads, d=128,
RoPE + RMS-norm on q/k) distributed over 8 NeuronCores.

Sharding: core c = 4*b + g handles batch b and kv-group g (4 q-heads, 1 kv
head). Each core computes its q/k/v projections, RoPE + RMS, causal
attention in transposed-score layout, and a row-parallel partial o_proj
(wo rows for its heads). Host sums the 4 partials per batch (all-reduce
equivalent) and transposes back.

All matmuls run as float32r (full PE rate at free-dim 512, ~1.5e-4 rms rel
error for K=2048 contractions). Softmax needs no max-subtraction: q is
RMS-normalized and k's RMS factor is applied inside the exp via the
per-partition ACT scale, so scores/sqrt(128) are bounded by ~±11.3 and the
causal -10000 mask is realized as an exact multiplicative 0/1 mask on the
diagonal tiles (fully-masked tiles are skipped).
"""
import sys, os

sys.path.insert(0, "/opt/trn_rl_repo")
BISECT = os.environ.get("K_BISECT", "all")

import numpy as np
import concourse.bass as bass
import concourse.bass_isa as bass_isa
from concourse import bacc
import concourse.mybir as mybir
import concourse.tile as tile
from concourse.bass_utils import run_bass_kernel_spmd
from concourse.masks import make_identity
from contextlib import ExitStack

P = 128
B, T, E = 2, 2048, 2048
NH = 4            # q heads per core
D = 128           # head dim
DQ = NH * D       # per-core q width
DKV = 128         # per-core kv width
TT = 512          # t-tile (projection output / attention query tile)
NTT = T // TT     # 4
NJT = T // P      # 16 key tiles
NEC = E // P      # 16 contraction chunks
NQ = 8            # x chunks per t-tile in phase 1
EPS = 1e-6
ROPE_BASE = 100000.0
F32 = mybir.dt.float32
F32R = mybir.dt.float32r
AF = mybir.ActivationFunctionType

_cache = {}


def _build():
    nc = bacc.Bacc("TRN2", target_bir_lowering=False, debug=False)
    xT_d = nc.dram_tensor("xT", [E, T], F32R, kind="ExternalInput")
    wq_d = nc.dram_tensor("wq", [E, DQ], F32R, kind="ExternalInput")
    wk_d = nc.dram_tensor("wk", [E, DKV], F32R, kind="ExternalInput")
    wv_d = nc.dram_tensor("wv", [E, DKV], F32R, kind="ExternalInput")
    wo_d = nc.dram_tensor("wo", [DQ, E], F32R, kind="ExternalInput")
    cos_d = nc.dram_tensor("cosf", [P, T], F32, kind="ExternalInput")
    sin_d = nc.dram_tensor("sinf", [P, T], F32, kind="ExternalInput")
    msk_d = nc.dram_tensor("msk", [P, 4, TT], F32R, kind="ExternalInput")
    ones_d = nc.dram_tensor("ones", [P, 1], F32R, kind="ExternalInput")
    out_d = nc.dram_tensor("outT", [E, T], F32, kind="ExternalOutput")

    H = D // 2  # rope half

    with ExitStack() as ctx:
        tc = ctx.enter_context(tile.TileContext(nc))
        persist = ctx.enter_context(tc.tile_pool(name="persist", bufs=1))

        # ---- persistent tiles (both phases) ----
        msk_sb = persist.tile([P, 4, TT], F32R, tag="msk")
        qT_sb = persist.tile([P, NH, T], F32R, tag="qT")
        kT_sb = persist.tile([P, T], F32R, tag="kT")
        v_sb = persist.tile([P, NJT, D], F32R, tag="v")
        ones_sb = persist.tile([P, 1], F32R, tag="ones")
        nc.sync.dma_start(ones_sb, ones_d.ap())
        epsq_sb = persist.tile([P, 1], F32, tag="epsq")
        nc.vector.memset(epsq_sb, EPS)
        ident_sb = persist.tile([P, P], F32, tag="ident")
        make_identity(nc, ident_sb)

        # ================= phase 1: projections + rope + rms =================
        with tc.tile_pool(name="wpool", bufs=1) as wpool, \
             tc.tile_pool(name="xpool", bufs=6) as xpool, \
             tc.tile_pool(name="csp", bufs=2) as csp, \
             tc.tile_pool(name="work1", bufs=3) as work1, \
             tc.tile_pool(name="rows1", bufs=3) as rows1, \
             tc.tile_pool(name="ps_q", bufs=7, space="PSUM") as ps_q, \
             tc.tile_pool(name="ps_vt", bufs=1, space="PSUM") as ps_vt:

            wq_sb = wpool.tile([P, NEC, DQ], F32R, tag="wq")
            wk_sb = wpool.tile([P, NEC, DKV], F32R, tag="wk")
            wv_sb = wpool.tile([P, NEC, DKV], F32R, tag="wv")
            for w_sb, w_d in ((wq_sb, wq_d), (wk_sb, wk_d), (wv_sb, wv_d)):
                wr = w_d.ap().rearrange("(c p) m -> p c m", p=P)
                for cc in range(0, NEC, 4):
                    nc.sync.dma_start(w_sb[:, cc : cc + 4, :], wr[:, cc : cc + 4, :])

            def rope(dst, psum, ctt, stt):
                rp = work1.tile([P, TT], F32, tag="rp")
                rt = work1.tile([P, TT], F32, tag="rt")
                nc.vector.tensor_mul(rp, psum, ctt)
                nc.vector.tensor_mul(rt[0:H], psum[H:D], stt[0:H])
                nc.vector.tensor_mul(rt[H:D], psum[0:H], stt[H:D])
                nc.vector.tensor_add(dst, rp, rt)
                return rp

            for tt in range(NTT):
                ts_ = slice(tt * TT, (tt + 1) * TT)
                ctt = csp.tile([P, TT], F32, tag="ctt")
                stt = csp.tile([P, TT], F32, tag="stt")
                nc.sync.dma_start(ctt, cos_d.ap()[:, ts_])
                nc.sync.dma_start(stt, sin_d.ap()[:, ts_])

                # 6 co-accumulating chains: 4 q heads, k, v
                chains = [(wq_sb, h * D, (h + 1) * D) for h in range(NH)]
                chains.append((wk_sb, 0, DKV))
                chains.append((wv_sb, 0, DKV))
                pts = [ps_q.tile([P, TT], F32, tag="q", name=f"pq{i}")
                       for i in range(6)]
                EC4 = NEC // NQ
                for c4 in range(NQ):
                    xq = xpool.tile([P, EC4, TT], F32R, tag="xh")
                    nc.sync.dma_start(
                        xq,
                        xT_d.ap()[c4 * (E // NQ) : (c4 + 1) * (E // NQ), ts_]
                        .rearrange("(c p) t -> p c t", p=P))
                    for ci, (w_sb, lo, hi) in enumerate(chains):
                        for c in range(EC4):
                            ec = c4 * EC4 + c
                            nc.tensor.matmul(pts[ci], w_sb[:, ec, lo:hi],
                                             xq[:, c, :], start=(ec == 0),
                                             stop=(ec == NEC - 1))

                # q heads and k share the same rms+rope pipeline
                dsts = [qT_sb[:, h, ts_] for h in range(NH)] + [kT_sb[:, ts_]]
                for ci, dst in enumerate(dsts):
                    pq = pts[ci]
                    # ACT drains psum (copy + square) so the psum slot frees
                    # fast and all DVE rope ops run SBUF-only (2x mode)
                    pqs = work1.tile([P, TT], F32, tag="pqs")
                    nc.scalar.copy(pqs, pq)
                    qsq = work1.tile([P, TT], F32, tag="qsq")
                    nc.scalar.square(qsq, pq)
                    # rms factor from pre-rope values (rope preserves norms)
                    ssb = work1.tile([P, TT], F32, tag="ssb")
                    nc.gpsimd.partition_all_reduce(ssb, qsq, channels=P,
                                                   reduce_op=bass_isa.ReduceOp.add)
                    srow = rows1.tile([1, TT], F32, tag="srow")
                    nc.scalar.activation(srow, ssb[0:1, :], AF.Sqrt,
                                         bias=epsq_sb[0:1, :], scale=1.0 / D)
                    rrow = rows1.tile([1, TT], F32, tag="rrow")
                    nc.vector.reciprocal(rrow, srow)
                    rbc = work1.tile([P, TT], F32, tag="rbc")
                    nc.gpsimd.partition_broadcast(rbc, rrow)
                    rp = work1.tile([P, TT], F32, tag="rp")
                    rt = work1.tile([P, TT], F32, tag="rt")
                    nc.vector.tensor_mul(rp, pqs, ctt)
                    nc.vector.tensor_mul(rt[0:H], pqs[H:D], stt[H:D])
                    nc.vector.tensor_mul(rt[H:D], pqs[0:H], stt[0:H])
                    nc.vector.tensor_add(rp, rp, rt)
                    nc.vector.tensor_mul(dst, rp, rbc)

                # v: PE-transpose into [t, d] chunks
                pv = pts[5]
                vtmp = work1.tile([P, TT], F32, tag="vtmp")
                nc.scalar.copy(vtmp, pv)
                for j4 in range(TT // P):
                    pvt = ps_vt.tile([P, P], F32, tag="vt")
                    nc.tensor.transpose(pvt, vtmp[:, j4 * P : (j4 + 1) * P],
                                        ident_sb)
                    nc.vector.tensor_copy(v_sb[:, tt * 4 + j4, :], pvt)

        # ================= phase 2+3: attention + o_proj =================
        if BISECT == "p1":
            nc.compile()
            return nc
        with tc.tile_pool(name="wop", bufs=1) as wop, \
             tc.tile_pool(name="ytp", bufs=2) as ytp, \
             tc.tile_pool(name="expp", bufs=6) as expp, \
             tc.tile_pool(name="work2", bufs=2) as work2, \
             tc.tile_pool(name="rows2", bufs=2) as rows2, \
             tc.tile_pool(name="ps_s", bufs=4, space="PSUM") as ps_s, \
             tc.tile_pool(name="ps_y", bufs=2, space="PSUM") as ps_y, \
             tc.tile_pool(name="ps_d", bufs=1, space="PSUM") as ps_d, \
             tc.tile_pool(name="ps_o", bufs=1, space="PSUM") as ps_o:

            nc.sync.dma_start(msk_sb, msk_d.ap())
            wo_sb = wop.tile([P, DQ // P, E], F32R, tag="wo")
            nc.sync.dma_start(wo_sb, wo_d.ap().rearrange("(c p) e -> p c e", p=P))

            for it in range(NTT):
                its = slice(it * TT, (it + 1) * TT)
                yt = ytp.tile([P, NH, TT], F32R, tag="yt")
                for h in range(NH):
                    py = ps_y.tile([P, TT], F32, tag="y")
                    pd = ps_d.tile([1, TT], F32, tag="d")
                    jlast = 4 * it + 3
                    for jt in range(jlast + 1):
                        pss_ = ps_s.tile([P, TT], F32, tag="s")
                        nc.tensor.matmul(pss_, kT_sb[:, jt * P : (jt + 1) * P],
                                         qT_sb[:, h, its], start=True, stop=True)
                        ex = expp.tile([P, TT], F32R, tag="ex")
                        nc.scalar.activation(ex, pss_, AF.Exp,
                                             scale=rstdk_sb[:, jt : jt + 1])
                        if jt >= 4 * it:
                            # diagonal tile: cols < 128*o are fully masked,
                            # cols >= 128*(o+1) fully valid; only the 128-wide
                            # window straddles the diagonal
                            o_ = jt - 4 * it
                            if o_ > 0:
                                nc.vector.tensor_scalar_mul(
                                    ex[:, 0 : P * o_], ex[:, 0 : P * o_], 0.0)
                            nc.vector.tensor_mul(
                                ex[:, P * o_ : P * (o_ + 1)],
                                ex[:, P * o_ : P * (o_ + 1)],
                                msk_sb[:, 0, 0:P])
                        nc.tensor.matmul(py, v_sb[:, jt, :], ex,
                                         start=(jt == 0), stop=(jt == jlast))
                        nc.tensor.matmul(pd, ones_sb, ex,
                                         start=(jt == 0), stop=(jt == jlast))
                    rd = rows2.tile([1, TT], F32, tag="rd")
                    nc.vector.reciprocal(rd, pd)
                    rdb = work2.tile([P, TT], F32, tag="rdb")
                    nc.gpsimd.partition_broadcast(rdb, rd)
                    nc.vector.tensor_mul(yt[:, h, :], py, rdb)
                for e in range(NEC):
                    po = ps_o.tile([P, TT], F32, tag="o")
                    for c in range(DQ // P):
                        nc.tensor.matmul(po, wo_sb[:, c, e * P : (e + 1) * P],
                                         yt[:, c, :], start=(c == 0),
                                         stop=(c == DQ // P - 1))
                    nc.sync.dma_start(out_d.ap()[e * P : (e + 1) * P, its], po)

    nc.compile()
    return nc


def _tables():
    half = D // 2
    inv_freq = 1.0 / (ROPE_BASE ** (np.arange(half, dtype=np.float64) / half))
    freqs = np.arange(T, dtype=np.float64)[:, None] * inv_freq[None, :]  # [T, half]
    cosT = np.cos(freqs).T.astype(np.float32)  # [half, T]
    sinT = np.sin(freqs).T.astype(np.float32)
    cos_full = np.concatenate([cosT, cosT], axis=0)          # [P, T]
    # rows 0:64 hold -sinT (multiplies x1 into out[64:128]), rows 64:128
    # hold +sinT (multiplies x2 into out[0:64]) so DVE input base
    # partitions match the swapped-half reads.
    sin_signed = np.concatenate([-sinT, sinT], axis=0)       # [P, T]
    # diagonal-tile masks: mask[p, o, i] = 1 if i >= p + 128*o
    i_idx = np.arange(TT)[None, None, :]
    p_idx = np.arange(P)[:, None, None]
    o_idx = np.arange(4)[None, :, None]
    msk = (i_idx >= p_idx + P * o_idx).astype(np.float32)    # [P, 4, TT]
    return cos_full, sin_signed, msk


def kernel(x, wq, wk, wv, wo):
    if "nc" not in _cache:
        _cache["nc"] = _build()
    nc = _cache["nc"]
    cos_full, sin_signed, msk = _tables()
    xTs = [np.ascontiguousarray(np.asarray(x)[b].T).astype(np.float32)
           for b in range(B)]
    wq, wk, wv, wo = (np.asarray(a, dtype=np.float32) for a in (wq, wk, wv, wo))
    in_maps = []
    for c in range(8):
        b, g = divmod(c, 4)
        in_maps.append({
            "xT": xTs[b],
            "wq": np.ascontiguousarray(wq[:, g * DQ : (g + 1) * DQ]),
            "wk": np.ascontiguousarray(wk[:, g * DKV : (g + 1) * DKV]),
            "wv": np.ascontiguousarray(wv[:, g * DKV : (g + 1) * DKV]),
            "wo": np.ascontiguousarray(wo[g * DQ : (g + 1) * DQ, :]),
            "cosf": cos_full,
            "sinf": sin_signed,
            "msk": msk,
            "ones": np.ones((P, 1), dtype=np.float32),
        })
    res = run_bass_kernel_spmd(nc, in_maps, core_ids=list(range(8)))
    out = np.zeros((B, T, E), dtype=np.float32)
    for c in range(8):
        b = c // 4
        out[b] += res.results[c]["outT"].T
    return out


# revision 26
# speedup vs baseline: 1.0052x; 1.0052x over previous
"""Causal GQA self-attention (b=2, t=2048, 16 q-heads / 4 kv-heads, d=128,
RoPE + RMS-norm on q/k) distributed over 8 NeuronCores.

Sharding: core c = 4*b + g handles batch b and kv-group g (4 q-heads, 1 kv
head). Each core computes its q/k/v projections, RoPE + RMS, causal
attention in transposed-score layout, and a row-parallel partial o_proj
(wo rows for its heads). Host sums the 4 partials per batch (all-reduce
equivalent) and transposes back.

All matmuls run as float32r (full PE rate at free-dim 512, ~1.5e-4 rms rel
error for K=2048 contractions). Softmax needs no max-subtraction: q is
RMS-normalized and k's RMS factor is applied inside the exp via the
per-partition ACT scale, so scores/sqrt(128) are bounded by ~±11.3 and the
causal -10000 mask is realized as an exact multiplicative 0/1 mask on the
diagonal tiles (fully-masked tiles are skipped).
"""
import sys, os

sys.path.insert(0, "/opt/trn_rl_repo")
BISECT = os.environ.get("K_BISECT", "all")

import numpy as np
import concourse.bass as bass
import concourse.bass_isa as bass_isa
from concourse import bacc
import concourse.mybir as mybir
import concourse.tile as tile
from concourse.bass_utils import run_bass_kernel_spmd
from concourse.masks import make_identity
from contextlib import ExitStack

P = 128
B, T, E = 2, 2048, 2048
NH = 4            # q heads per core
D = 128           # head dim
DQ = NH * D       # per-core q width
DKV = 128         # per-core kv width
TT = 512          # t-tile (projection output / attention query tile)
NTT = T // TT     # 4
NJT = T // P      # 16 key tiles
NEC = E // P      # 16 contraction chunks
NQ = 8            # x chunks per t-tile in phase 1
EPS = 1e-6
ROPE_BASE = 100000.0
F32 = mybir.dt.float32
F32R = mybir.dt.float32r
AF = mybir.ActivationFunctionType

_cache = {}


def _build():
    nc = bacc.Bacc("TRN2", target_bir_lowering=False, debug=False)
    xT_d = nc.dram_tensor("xT", [E, T], F32R, kind="ExternalInput")
    wq_d = nc.dram_tensor("wq", [E, DQ], F32R, kind="ExternalInput")
    wk_d = nc.dram_tensor("wk", [E, DKV], F32R, kind="ExternalInput")
    wv_d = nc.dram_tensor("wv", [E, DKV], F32R, kind="ExternalInput")
    wo_d = nc.dram_tensor("wo", [DQ, E], F32R, kind="ExternalInput")
    cos_d = nc.dram_tensor("cosf", [P, T], F32, kind="ExternalInput")
    sin_d = nc.dram_tensor("sinf", [P, T], F32, kind="ExternalInput")
    msk_d = nc.dram_tensor("msk", [P, 4, TT], F32R, kind="ExternalInput")
    ones_d = nc.dram_tensor("ones", [P, 1], F32R, kind="ExternalInput")
    out_d = nc.dram_tensor("outT", [E, T], F32, kind="ExternalOutput")

    H = D // 2  # rope half

    with ExitStack() as ctx:
        tc = ctx.enter_context(tile.TileContext(nc))
        persist = ctx.enter_context(tc.tile_pool(name="persist", bufs=1))

        # ---- persistent tiles (both phases) ----
        msk_sb = persist.tile([P, 4, TT], F32R, tag="msk")
        qT_sb = persist.tile([P, NH, T], F32R, tag="qT")
        kT_sb = persist.tile([P, T], F32R, tag="kT")
        v_sb = persist.tile([P, NJT, D], F32R, tag="v")
        ones_sb = persist.tile([P, 1], F32R, tag="ones")
        nc.sync.dma_start(ones_sb, ones_d.ap())
        epsq_sb = persist.tile([P, 1], F32, tag="epsq")
        nc.vector.memset(epsq_sb, EPS)
        ident_sb = persist.tile([P, P], F32, tag="ident")
        make_identity(nc, ident_sb)

        # ================= phase 1: projections + rope + rms =================
        with tc.tile_pool(name="wpool", bufs=1) as wpool, \
             tc.tile_pool(name="xpool", bufs=6) as xpool, \
             tc.tile_pool(name="csp", bufs=2) as csp, \
             tc.tile_pool(name="work1", bufs=4) as work1, \
             tc.tile_pool(name="rows1", bufs=3) as rows1, \
             tc.tile_pool(name="ps_q", bufs=7, space="PSUM") as ps_q, \
             tc.tile_pool(name="ps_vt", bufs=1, space="PSUM") as ps_vt:

            wq_sb = wpool.tile([P, NEC, DQ], F32R, tag="wq")
            wk_sb = wpool.tile([P, NEC, DKV], F32R, tag="wk")
            wv_sb = wpool.tile([P, NEC, DKV], F32R, tag="wv")
            for w_sb, w_d in ((wq_sb, wq_d), (wk_sb, wk_d), (wv_sb, wv_d)):
                wr = w_d.ap().rearrange("(c p) m -> p c m", p=P)
                for cc in range(0, NEC, 4):
                    nc.sync.dma_start(w_sb[:, cc : cc + 4, :], wr[:, cc : cc + 4, :])

            def rope(dst, psum, ctt, stt):
                rp = work1.tile([P, TT], F32, tag="rp")
                rt = work1.tile([P, TT], F32, tag="rt")
                nc.vector.tensor_mul(rp, psum, ctt)
                nc.vector.tensor_mul(rt[0:H], psum[H:D], stt[0:H])
                nc.vector.tensor_mul(rt[H:D], psum[0:H], stt[H:D])
                nc.vector.tensor_add(dst, rp, rt)
                return rp

            for tt in range(NTT):
                ts_ = slice(tt * TT, (tt + 1) * TT)
                ctt = csp.tile([P, TT], F32, tag="ctt")
                stt = csp.tile([P, TT], F32, tag="stt")
                nc.sync.dma_start(ctt, cos_d.ap()[:, ts_])
                nc.sync.dma_start(stt, sin_d.ap()[:, ts_])

                # 6 co-accumulating chains: 4 q heads, k, v
                chains = [(wq_sb, h * D, (h + 1) * D) for h in range(NH)]
                chains.append((wk_sb, 0, DKV))
                chains.append((wv_sb, 0, DKV))
                pts = [ps_q.tile([P, TT], F32, tag="q", name=f"pq{i}")
                       for i in range(6)]
                EC4 = NEC // NQ
                for c4 in range(NQ):
                    xq = xpool.tile([P, EC4, TT], F32R, tag="xh")
                    nc.sync.dma_start(
                        xq,
                        xT_d.ap()[c4 * (E // NQ) : (c4 + 1) * (E // NQ), ts_]
                        .rearrange("(c p) t -> p c t", p=P))
                    for ci, (w_sb, lo, hi) in enumerate(chains):
                        for c in range(EC4):
                            ec = c4 * EC4 + c
                            nc.tensor.matmul(pts[ci], w_sb[:, ec, lo:hi],
                                             xq[:, c, :], start=(ec == 0),
                                             stop=(ec == NEC - 1))

                # q heads and k share the same rms+rope pipeline
                dsts = [qT_sb[:, h, ts_] for h in range(NH)] + [kT_sb[:, ts_]]
                for ci, dst in enumerate(dsts):
                    pq = pts[ci]
                    # ACT drains psum (copy + square) so the psum slot frees
                    # fast and all DVE rope ops run SBUF-only (2x mode)
                    pqs = work1.tile([P, TT], F32, tag="pqs")
                    nc.scalar.copy(pqs, pq)
                    qsq = work1.tile([P, TT], F32, tag="qsq")
                    nc.scalar.square(qsq, pq)
                    # rms factor from pre-rope values (rope preserves norms)
                    ssb = work1.tile([P, TT], F32, tag="ssb")
                    nc.gpsimd.partition_all_reduce(ssb, qsq, channels=P,
                                                   reduce_op=bass_isa.ReduceOp.add)
                    srow = rows1.tile([1, TT], F32, tag="srow")
                    nc.scalar.activation(srow, ssb[0:1, :], AF.Sqrt,
                                         bias=epsq_sb[0:1, :], scale=1.0 / D)
                    rrow = rows1.tile([1, TT], F32, tag="rrow")
                    nc.vector.reciprocal(rrow, srow)
                    rbc = work1.tile([P, TT], F32, tag="rbc")
                    nc.gpsimd.partition_broadcast(rbc, rrow)
                    rp = work1.tile([P, TT], F32, tag="rp")
                    rt = work1.tile([P, TT], F32, tag="rt")
                    nc.vector.tensor_mul(rp, pqs, ctt)
                    nc.vector.tensor_mul(rt[0:H], pqs[H:D], stt[H:D])
                    nc.vector.tensor_mul(rt[H:D], pqs[0:H], stt[0:H])
                    nc.vector.tensor_add(rp, rp, rt)
                    nc.vector.tensor_mul(dst, rp, rbc)

                # v: PE-transpose into [t, d] chunks
                pv = pts[5]
                vtmp = work1.tile([P, TT], F32, tag="vtmp")
                nc.scalar.copy(vtmp, pv)
                for j4 in range(TT // P):
                    pvt = ps_vt.tile([P, P], F32, tag="vt")
                    nc.tensor.transpose(pvt, vtmp[:, j4 * P : (j4 + 1) * P],
                                        ident_sb)
                    nc.vector.tensor_copy(v_sb[:, tt * 4 + j4, :], pvt)

        # ================= phase 2+3: attention + o_proj =================
        if BISECT == "p1":
            nc.compile()
            return nc
        with tc.tile_pool(name="wop", bufs=1) as wop, \
             tc.tile_pool(name="ytp", bufs=2) as ytp, \
             tc.tile_pool(name="expp", bufs=6) as expp, \
             tc.tile_pool(name="work2", bufs=3) as work2, \
             tc.tile_pool(name="rows2", bufs=2) as rows2, \
             tc.tile_pool(name="ps_s", bufs=4, space="PSUM") as ps_s, \
             tc.tile_pool(name="ps_y", bufs=2, space="PSUM") as ps_y, \
             tc.tile_pool(name="ps_d", bufs=1, space="PSUM") as ps_d, \
             tc.tile_pool(name="ps_o", bufs=1, space="PSUM") as ps_o:

            nc.sync.dma_start(msk_sb, msk_d.ap())
            wo_sb = wop.tile([P, DQ // P, E], F32R, tag="wo")
            nc.sync.dma_start(wo_sb, wo_d.ap().rearrange("(c p) e -> p c e", p=P))

            for it in range(NTT):
                its = slice(it * TT, (it + 1) * TT)
                yt = ytp.tile([P, NH, TT], F32R, tag="yt")
                for h in range(NH):
                    py = ps_y.tile([P, TT], F32, tag="y")
                    pd = ps_d.tile([1, TT], F32, tag="d")
                    jlast = 4 * it + 3
                    for jt in range(jlast + 1):
                        pss_ = ps_s.tile([P, TT], F32, tag="s")
                        nc.tensor.matmul(pss_, kT_sb[:, jt * P : (jt + 1) * P],
                                         qT_sb[:, h, its], start=True, stop=True)
                        ex = expp.tile([P, TT], F32R, tag="ex")
                        nc.scalar.activation(ex, pss_, AF.Exp,
                                             scale=rstdk_sb[:, jt : jt + 1])
                        if jt >= 4 * it:
                            # diagonal tile: cols < 128*o are fully masked,
                            # cols >= 128*(o+1) fully valid; only the 128-wide
                            # window straddles the diagonal
                            o_ = jt - 4 * it
                            if o_ > 0:
                                nc.vector.tensor_scalar_mul(
                                    ex[:, 0 : P * o_], ex[:, 0 : P * o_], 0.0)
                            nc.vector.tensor_mul(
                                ex[:, P * o_ : P * (o_ + 1)],
                                ex[:, P * o_ : P * (o_ + 1)],
                                msk_sb[:, 0, 0:P])
                        nc.tensor.matmul(py, v_sb[:, jt, :], ex,
                                         start=(jt == 0), stop=(jt == jlast))
                        nc.tensor.matmul(pd, ones_sb, ex,
                                         start=(jt == 0), stop=(jt == jlast))
                    rd = rows2.tile([1, TT], F32, tag="rd")
                    nc.vector.reciprocal(rd, pd)
                    rdb = work2.tile([P, TT], F32, tag="rdb")
                    nc.gpsimd.partition_broadcast(rdb, rd)
                    nc.vector.tensor_mul(yt[:, h, :], py, rdb)
                for e in range(NEC):
                    po = ps_o.tile([P, TT], F32, tag="o")
                    for c in range(DQ // P):
                        nc.tensor.matmul(po, wo_sb[:, c, e * P : (e + 1) * P],
                                         yt[:, c, :], start=(c == 0),
                                         stop=(c == DQ // P - 1))
                    nc.sync.dma_start(out_d.ap()[e * P : (e + 1) * P, its], po)

    nc.compile()
    return nc


def _tables():
    half = D // 2
    inv_freq = 1.0 / (ROPE_BASE ** (np.arange(half, dtype=np.float64) / half))
    freqs = np.arange(T, dtype=np.float64)[:, None] * inv_freq[None, :]  # [T, half]
    cosT = np.cos(freqs).T.astype(np.float32)  # [half, T]
    sinT = np.sin(freqs).T.astype(np.float32)
    cos_full = np.concatenate([cosT, cosT], axis=0)          # [P, T]
    # rows 0:64 hold -sinT (multiplies x1 into out[64:128]), rows 64:128
    # hold +sinT (multiplies x2 into out[0:64]) so DVE input base
    # partitions match the swapped-half reads.
    sin_signed = np.concatenate([-sinT, sinT], axis=0)       # [P, T]
    # diagonal-tile masks: mask[p, o, i] = 1 if i >= p + 128*o
    i_idx = np.arange(TT)[None, None, :]
    p_idx = np.arange(P)[:, None, None]
    o_idx = np.arange(4)[None, :, None]
    msk = (i_idx >= p_idx + P * o_idx).astype(np.float32)    # [P, 4, TT]
    return cos_full, sin_signed, msk


def kernel(x, wq, wk, wv, wo):
    if "nc" not in _cache:
        _cache["nc"] = _build()
    nc = _cache["nc"]
    cos_full, sin_signed, msk = _tables()
    xTs = [np.ascontiguousarray(np.asarray(x)[b].T).astype(np.float32)
           for b in range(B)]
    wq, wk, wv, wo = (np.asarray(a, dtype=np.float32) for a in (wq, wk, wv, wo))
    in_maps = []
    for c in range(8):
        b, g = divmod(c, 4)
        in_maps.append({
            "xT": xTs[b],
            "wq": np.ascontiguousarray(wq[:, g * DQ : (g + 1) * DQ]),
            "wk": np.ascontiguousarray(wk[:, g * DKV : (g + 1) * DKV]),
            "wv": np.ascontiguousarray(wv[:, g * DKV : (g + 1) * DKV]),
            "wo": np.ascontiguousarray(wo[g * DQ : (g + 1) * DQ, :]),
            "cosf": cos_full,
            "sinf": sin_signed,
            "msk": msk,
            "ones": np.ones((P, 1), dtype=np.float32),
        })
    res = run_bass_kernel_spmd(nc, in_maps, core_ids=list(range(8)))
    out = np.zeros((B, T, E), dtype=np.float32)
    for c in range(8):
        b = c // 4
        out[b] += res.results[c]["outT"].T
    return out
